# revision 1
# baseline (speedup 1.0000x reference)
"""Trainium2 Bass kernel: batched causal single-head self-attention.

Reference computation (per batch b):
    q = x @ Wq; k = x @ Wk; v = x @ Wv          # [T, H] each, contraction over E
    S = (q @ k^T) / sqrt(H)                     # [T, T]
    P = softmax(causal_mask(S), axis=-1)
    out = P @ v                                 # [T, H]

Shapes: x [512, 256, 384] f32, W* [384, 64] f32, out [512, 256, 64] f32.
Sharding: pure data parallel, 64 batches per NeuronCore across 8 cores.

Device algorithm per batch (all matmul operands bf16, fp32 PSUM accumulation):
  - host ships xT = x^T per batch ([E, T] layout, E on partitions) so every
    matmul has its contraction dim on partitions.
  - qk^T = [Wq|Wk]^T @ xT      (one packed 128-wide stationary, 3 E-chunks)
  - v^T  = Wv^T @ xT           (3 E-chunks)
  - v    = PE-transpose of v^T, with a ones row appended so the transposed
    tile carries a ones column (used to get softmax denominators for free)
  - S^T  = k^T.T @ q^T         ([tk, tq] layout; lower-left T/4 block skipped)
  - P    = exp(0.125 * S^T)    (ScalarE; no max-subtraction needed, |s|<~45)
  - P   *= causal 0/1 mask     (multiplicative, on the two diagonal blocks)
  - outA = P^T-contracted V: out_aug[tq, 0:65] = sum_tk P[tk,tq] * [v|1][tk]
    so col 64 = softmax denominator; divide + store.
"""

import numpy as np
import ml_dtypes

B, T, E, H = 512, 256, 384, 64
NCORES = 8
BPC = B // NCORES  # 64
P = 128
EC = E // P  # 3
HP1 = H + 1  # 65

_cache: dict = {}


def _install_ntff_hook():
    """Shim antenv.axon_hooks (absent in this image) so run_bass_kernel_spmd
    trace=True can capture NTFF profiles via the axon .so's C ABI."""
    import contextlib
    import ctypes
    import sys
    import types

    if "antenv.axon_hooks" in sys.modules:
        return
    so_path = "/opt/axon/libaxon_pjrt.so"
    lib = ctypes.CDLL(so_path)
    if not hasattr(lib, "axon_start_nrt_profile"):
        return
    lib.axon_start_nrt_profile.argtypes = [
        ctypes.POINTER(ctypes.c_int64),
        ctypes.c_size_t,
    ]
    lib.axon_start_nrt_profile.restype = ctypes.c_int64
    lib.axon_stop_nrt_profile.argtypes = [ctypes.c_char_p]
    lib.axon_stop_nrt_profile.restype = ctypes.c_int64

    @contextlib.contextmanager
    def _hook(output_dir, device_ids):
        import jax

        jax.devices()
        if device_ids:
            ids = (ctypes.c_int64 * len(device_ids))(*device_ids)
            rc = lib.axon_start_nrt_profile(ids, len(device_ids))
        else:
            rc = lib.axon_start_nrt_profile(None, 0)
        if rc != 0:
            raise RuntimeError(f"axon_start_nrt_profile rc={rc}")
        try:
            yield
        finally:
            n = lib.axon_stop_nrt_profile(str(output_dir).encode())
            if n < 0:
                raise RuntimeError(f"axon_stop_nrt_profile rc={n}")
            print(f"profile: {n} file(s) written to {output_dir}", file=sys.stderr)

    mod = types.ModuleType("antenv.axon_hooks")
    _state = {"hook": _hook}
    mod.get_axon_ntff_profile_hook = lambda: _state["hook"]
    mod.set_axon_ntff_profile_hook = lambda h: _state.__setitem__("hook", h)
    sys.modules["antenv.axon_hooks"] = mod


def _build_program(bpc):
    import concourse.bacc as bacc
    import concourse.mybir as mybir
    import concourse.tile as tile

    f32 = mybir.dt.float32
    bf16 = mybir.dt.bfloat16
    Exp = mybir.ActivationFunctionType.Exp
    Mult = mybir.AluOpType.mult

    nc = bacc.Bacc(
        "TRN2",
        target_bir_lowering=False,
        debug=False,
        enable_asserts=False,
        num_devices=NCORES,
    )
    xt_d = nc.dram_tensor("xt", [bpc, P, EC, T], bf16, kind="ExternalInput").ap()
    wqk_d = nc.dram_tensor("wqk", [P, EC, P], bf16, kind="ExternalInput").ap()
    wv_d = nc.dram_tensor("wv", [P, EC, H], bf16, kind="ExternalInput").ap()
    # ones-padded multiplicative causal mask for P^T tiles:
    # cols 0:128 = tril01 (tk<=tq), cols 128:256 = 1, cols 256:384 = tril01
    um_d = nc.dram_tensor("um", [P, 3 * P], bf16, kind="ExternalInput").ap()
    iden_d = nc.dram_tensor("iden", [HP1, HP1], bf16, kind="ExternalInput").ap()
    out_d = nc.dram_tensor("out", [bpc, T, H], f32, kind="ExternalOutput").ap()

    # DMA instructions have a flat ~600ns issue cost on the Sync sequencer, so
    # in/out traffic is batched in groups of Q=4 batches; compute + PSUM ops
    # stay pair-granular.
    Q = 4
    assert bpc % Q == 0
    nquads = bpc // Q

    with tile.TileContext(nc) as tc:
        with (
            tc.tile_pool(name="const", bufs=1) as constp,
            tc.tile_pool(name="xin", bufs=4) as xpool,
            tc.tile_pool(name="qksb", bufs=3) as qkpool,
            tc.tile_pool(name="ksh", bufs=3) as kpool,
            tc.tile_pool(name="psb", bufs=3) as ppool,
            tc.tile_pool(name="vaug", bufs=3) as vpool,
            tc.tile_pool(name="osb", bufs=2) as opool,
            tc.tile_pool(name="rec", bufs=2) as rpool,
            tc.tile_pool(name="ps_qk", bufs=2, space="PSUM") as ps_qk,
            tc.tile_pool(name="ps_vt", bufs=1, space="PSUM") as ps_vt,
            tc.tile_pool(name="ps_s", bufs=3, space="PSUM") as ps_s,
            tc.tile_pool(name="ps_tr", bufs=1, space="PSUM") as ps_tr,
            tc.tile_pool(name="ps_o", bufs=1, space="PSUM") as ps_o,
        ):
            wqk = constp.tile([P, EC, P], bf16)
            nc.sync.dma_start(wqk, wqk_d)
            wv = constp.tile([P, EC, H], bf16)
            nc.sync.dma_start(wv, wv_d)
            um = constp.tile([P, 3 * P], bf16)
            nc.sync.dma_start(um, um_d)
            iden = constp.tile([HP1, HP1], bf16)
            nc.sync.dma_start(iden, iden_d)
            # v^T staging with a persistent ones row at partition 64 (manual
            # double-buffer so the ones row survives across iterations)
            vtabs = []
            for i in range(2):
                vt = constp.tile([HP1, 2, T], bf16, name=f"vtab{i}")
                nc.vector.memset(vt[H : H + 1, :, :], 1.0)
                vtabs.append(vt)
            # k^T staging padded to 128 partitions with zero rows 64:128 so the
            # scores matmuls use full-width stationaries (FWL) and stream q^T
            # directly from qk_sb (zero k rows null out the garbage rows)
            kabs = []
            for i in range(2):
                kt = constp.tile([P, Q, T], bf16, name=f"kab{i}")
                nc.vector.memset(kt[H:P], 0.0)
                kabs.append(kt)
            # persistent transpose PSUM tiles: pad columns stay zero so the
            # v_aug cast can read the full tile contiguously

            for qd in range(nquads):
                b0 = Q * qd
                xt = xpool.tile([P, Q, EC, T], bf16)
                nc.sync.dma_start(
                    xt, xt_d[b0 : b0 + Q].rearrange("s p c t -> p s c t")
                )
                qk_sb = qkpool.tile([P, Q, T], bf16)
                k_sb = kabs[qd % 2]
                o_sb = opool.tile([P, Q, 2, H], f32)

                v_augs = []
                for prl in range(Q // 2):
                    s0 = 2 * prl
                    pr = qd * (Q // 2) + prl

                    qk_ps = ps_qk.tile([P, 2, T], f32)
                    vt_ps = ps_vt.tile([H, 2, T], f32)
                    for s in range(2):
                        for c in range(EC):
                            nc.tensor.matmul(
                                qk_ps[:, s, :],
                                wqk[:, c, :],
                                xt[:, s0 + s, c, :],
                                start=(c == 0),
                                stop=(c == EC - 1),
                            )
                    for s in range(2):
                        for c in range(EC):
                            nc.tensor.matmul(
                                vt_ps[:, s, :],
                                wv[:, c, :],
                                xt[:, s0 + s, c, :],
                                start=(c == 0),
                                stop=(c == EC - 1),
                            )

                    # [q^T; k^T] PSUM -> SBUF bf16 (ScalarE, exp-set Copy)
                    nc.scalar.copy(qk_sb[:, s0 : s0 + 2, :], qk_ps)

                    vtab = vtabs[pr % 2]
                    nc.vector.tensor_copy(vtab[0:H], vt_ps)

                    tr_ps = ps_tr.tile([P, 2, 2, HP1 + 1], bf16)
                    for s in range(2):
                        for j in range(2):
                            nc.tensor.transpose(
                                tr_ps[:, s, j, 0:HP1],
                                vtab[:, s, j * P : (j + 1) * P],
                                iden,
                            )
                    v_aug = vpool.tile([P, 2, 2, HP1], bf16)
                    nc.vector.tensor_copy(v_aug, tr_ps[:, :, :, 0:HP1])
                    v_augs.append(v_aug)

                # k^T partitions 64:128 -> 0:64 (DMA shift), whole quad at once
                nc.sync.dma_start(k_sb[0:H], qk_sb[H:P])

                for prl in range(Q // 2):
                    s0 = 2 * prl
                    v_aug = v_augs[prl]

                    p_sb = ppool.tile([P, 2, 3 * P], bf16)
                    for s in range(2):
                        s_ps = ps_s.tile([P, 3 * P], f32, name="s_ps")
                        # S^T[tk 0:128, tq 0:256]
                        nc.tensor.matmul(
                            s_ps[:, 0:T],
                            k_sb[:, s0 + s, 0:P],
                            qk_sb[:, s0 + s, :],
                            start=True,
                            stop=True,
                        )
                        # S^T[tk 128:256, tq 128:256]
                        nc.tensor.matmul(
                            s_ps[:, T : 3 * P],
                            k_sb[:, s0 + s, P:T],
                            qk_sb[:, s0 + s, P:T],
                            start=True,
                            stop=True,
                        )
                        nc.scalar.activation(
                            p_sb[:, s, :], s_ps, Exp, scale=0.125
                        )

                    # multiplicative causal mask on both batches at once
                    nc.vector.tensor_tensor(
                        p_sb,
                        p_sb,
                        um[:, None, :].to_broadcast([P, 2, 3 * P]),
                        Mult,
                    )

                    o_ps = ps_o.tile([P, 2, 2, HP1], f32)
                    for s in range(2):
                        nc.tensor.matmul(
                            o_ps[:, s, 0, :],
                            p_sb[:, s, 0:P],
                            v_aug[:, s, 0, :],
                            start=True,
                            stop=True,
                        )
                        nc.tensor.matmul(
                            o_ps[:, s, 1, :],
                            p_sb[:, s, P:T],
                            v_aug[:, s, 0, :],
                            start=True,
                            stop=False,
                        )
                        nc.tensor.matmul(
                            o_ps[:, s, 1, :],
                            p_sb[:, s, T : 3 * P],
                            v_aug[:, s, 1, :],
                            start=False,
                            stop=True,
                        )

                    rec = rpool.tile([P, 2, 2, 1], f32)
                    nc.vector.reciprocal(rec, o_ps[:, :, :, H : H + 1])
                    nc.vector.tensor_tensor(
                        o_sb[:, s0 : s0 + 2, :, :],
                        o_ps[:, :, :, 0:H],
                        rec.to_broadcast([P, 2, 2, H]),
                        Mult,
                    )

                nc.sync.dma_start(
                    out_d[b0 : b0 + Q].rearrange("s (j p) h -> p s j h", p=P),
                    o_sb,
                )

    nc.compile()
    return nc


def _prep_inputs(x, Wq, Wk, Wv, bpc):
    bf = ml_dtypes.bfloat16
    nb = NCORES * bpc
    x = np.asarray(x, dtype=np.float32)[:nb]
    # [b, t, e] -> [b, p, c, t] with e = c*128 + p
    xt = np.ascontiguousarray(
        x.reshape(nb, T, EC, P).transpose(0, 3, 2, 1)
    ).astype(bf)
    wqk = np.concatenate(
        [np.asarray(Wq, np.float32), np.asarray(Wk, np.float32)], axis=1
    )  # [E, 128]
    wqk = np.ascontiguousarray(wqk.reshape(EC, P, P).transpose(1, 0, 2)).astype(bf)
    wv = np.ascontiguousarray(
        np.asarray(Wv, np.float32).reshape(EC, P, H).transpose(1, 0, 2)
    ).astype(bf)
    tril01 = (np.arange(P)[:, None] <= np.arange(P)[None, :]).astype(np.float32)
    um = np.concatenate([tril01, np.ones((P, P), np.float32), tril01], axis=1).astype(
        bf
    )
    iden = np.eye(HP1, dtype=np.float32).astype(bf)
    per_core = []
    for c in range(NCORES):
        per_core.append(
            {
                "xt": xt[c * bpc : (c + 1) * bpc],
                "wqk": wqk,
                "wv": wv,
                "um": um,
                "iden": iden,
            }
        )
    return per_core


def kernel(x, Wq, Wk, Wv, _trace=False, _bpc=BPC):
    """Full inputs in, full output out. Shards batch dim over 8 NeuronCores."""
    from concourse import bass_utils

    if _trace:
        _install_ntff_hook()

    key = ("prog", _bpc)
    if key not in _cache:
        _cache[key] = _build_program(_bpc)
    nc = _cache[key]

    in_maps = _prep_inputs(x, Wq, Wk, Wv, _bpc)
    res = bass_utils.run_bass_kernel_spmd(
        nc, in_maps, core_ids=list(range(NCORES)), trace=_trace
    )
    _cache["last_result"] = res
    out = np.concatenate([r["out"] for r in res.results], axis=0)
    return out.astype(np.float32)



# revision 6
# speedup vs baseline: 1.0279x; 1.0279x over previous
"""Trainium2 Bass kernel: batched causal single-head self-attention.

Reference computation (per batch b):
    q = x @ Wq; k = x @ Wk; v = x @ Wv          # [T, H] each, contraction over E
    S = (q @ k^T) / sqrt(H)                     # [T, T]
    P = softmax(causal_mask(S), axis=-1)
    out = P @ v                                 # [T, H]

Shapes: x [512, 256, 384] f32, W* [384, 64] f32, out [512, 256, 64] f32.
Sharding: pure data parallel, 64 batches per NeuronCore across 8 cores.

Device algorithm per batch (matmul operands bf16, fp32 PSUM accumulation):
  - host ships xt = x^T per batch ([E, T] layout, E on partitions, p-major
    DRAM so every DMA is one contiguous run per partition).
  - [q^T; k^T] = [Wq|Wk]^T @ xt     (one packed 128-wide stationary, 3
    E-chunks, both batches of a pair as one N=512 moving operand)
  - v        = xt_chunk.T @ Wv      (xt chunks [e,t] as stationary -> v in
    [t, h] layout directly; no transpose anywhere)
  - S^T      = k^T.T @ q^T          ([tk, tq]; lower-left T/4 block skipped)
  - P        = exp(0.125 * S^T)     (ScalarE; no max-subtraction, |s|<~45)
  - P       *= causal 0/1 mask      (GpSimd, only the two diagonal blocks)
  - out_aug[tq, 0:65] = sum_tk P[tk,tq] * [v|1][tk]  (col 64 = softmax
    denominator via the ones column); divide on DVE, store bf16.
"""

import numpy as np
import ml_dtypes

B, T, E, H = 512, 256, 384, 64
NCORES = 8
BPC = B // NCORES  # 64
P = 128
EC = E // P  # 3
HP1 = H + 1  # 65

_cache: dict = {}
import os
_MASK_ENGINE = os.environ.get("MASK_ENGINE", "gpsimd")


def _install_ntff_hook():
    """Shim antenv.axon_hooks (absent in this image) so run_bass_kernel_spmd
    trace=True can capture NTFF profiles via the axon .so's C ABI."""
    import contextlib
    import ctypes
    import sys
    import types

    if "antenv.axon_hooks" in sys.modules:
        return
    so_path = "/opt/axon/libaxon_pjrt.so"
    lib = ctypes.CDLL(so_path)
    if not hasattr(lib, "axon_start_nrt_profile"):
        return
    lib.axon_start_nrt_profile.argtypes = [
        ctypes.POINTER(ctypes.c_int64),
        ctypes.c_size_t,
    ]
    lib.axon_start_nrt_profile.restype = ctypes.c_int64
    lib.axon_stop_nrt_profile.argtypes = [ctypes.c_char_p]
    lib.axon_stop_nrt_profile.restype = ctypes.c_int64

    @contextlib.contextmanager
    def _hook(output_dir, device_ids):
        import jax

        jax.devices()
        if device_ids:
            ids = (ctypes.c_int64 * len(device_ids))(*device_ids)
            rc = lib.axon_start_nrt_profile(ids, len(device_ids))
        else:
            rc = lib.axon_start_nrt_profile(None, 0)
        if rc != 0:
            raise RuntimeError(f"axon_start_nrt_profile rc={rc}")
        try:
            yield
        finally:
            n = lib.axon_stop_nrt_profile(str(output_dir).encode())
            if n < 0:
                raise RuntimeError(f"axon_stop_nrt_profile rc={n}")
            print(f"profile: {n} file(s) written to {output_dir}", file=sys.stderr)

    mod = types.ModuleType("antenv.axon_hooks")
    _state = {"hook": _hook}
    mod.get_axon_ntff_profile_hook = lambda: _state["hook"]
    mod.set_axon_ntff_profile_hook = lambda h: _state.__setitem__("hook", h)
    sys.modules["antenv.axon_hooks"] = mod


def _build_program(bpc):
    import concourse.bacc as bacc
    import concourse.mybir as mybir
    import concourse.tile as tile

    f32 = mybir.dt.float32
    bf16 = mybir.dt.bfloat16
    Exp = mybir.ActivationFunctionType.Exp
    Mult = mybir.AluOpType.mult

    nc = bacc.Bacc(
        "TRN2",
        target_bir_lowering=False,
        debug=False,
        enable_asserts=False,
        num_devices=NCORES,
    )
    # p-major DRAM layouts: one contiguous run per partition per DMA.
    xt_d = nc.dram_tensor("xt", [P, bpc, EC, T], bf16, kind="ExternalInput").ap()
    wqk_d = nc.dram_tensor("wqk", [P, EC, P], bf16, kind="ExternalInput").ap()
    wv_d = nc.dram_tensor("wv", [P, EC, H], bf16, kind="ExternalInput").ap()
    tril_d = nc.dram_tensor("tril", [P, P], bf16, kind="ExternalInput").ap()
    out_d = nc.dram_tensor("out", [P, bpc, 2, H], bf16, kind="ExternalOutput").ap()
    _dbg = os.environ.get("DEBUG_TAPS") == "1"
    if _dbg:
        dbg_qk = nc.dram_tensor("dbg_qk", [P, bpc, T], bf16, kind="ExternalOutput").ap()
        dbg_k = nc.dram_tensor("dbg_k", [P, bpc, T], bf16, kind="ExternalOutput").ap()
        dbg_p = nc.dram_tensor("dbg_p", [P, bpc, 3 * P], bf16, kind="ExternalOutput").ap()
        dbg_v = nc.dram_tensor("dbg_v", [P, bpc, 2, HP1], bf16, kind="ExternalOutput").ap()
        dbg_o = nc.dram_tensor("dbg_o", [P, bpc, 2, HP1], f32, kind="ExternalOutput").ap()

    OC = 8  # batches per octet (DMA granularity)
    assert bpc % OC == 0
    nocts = bpc // OC

    with tile.TileContext(nc) as tc:
        with (
            tc.tile_pool(name="const", bufs=1) as constp,
            tc.tile_pool(name="xin", bufs=2) as xpool,
            tc.tile_pool(name="qksb", bufs=2) as qkpool,
            tc.tile_pool(name="psb", bufs=3) as ppool,
            tc.tile_pool(name="osb", bufs=2) as opool,
            tc.tile_pool(name="rec", bufs=2) as rpool,
            tc.tile_pool(name="ps_qk", bufs=2, space="PSUM") as ps_qk,
            tc.tile_pool(name="ps_v", bufs=2, space="PSUM") as ps_v,
            tc.tile_pool(name="ps_s", bufs=2, space="PSUM") as ps_s,
            tc.tile_pool(name="ps_o", bufs=2, space="PSUM") as ps_o,
        ):
            wqk = constp.tile([P, EC, P], bf16)
            nc.sync.dma_start(wqk, wqk_d)
            wv = constp.tile([P, EC, H], bf16)
            nc.sync.dma_start(wv, wv_d)
            tril = constp.tile([P, P], bf16)
            nc.sync.dma_start(tril, tril_d)
            trilb = tril[:, None, :].to_broadcast([P, 2, P])

            # k^T staging padded to 128 partitions with zero rows 64:128 so
            # the scores matmuls use full-width stationaries; shift-DMA fills
            # rows 0:64 each octet, the zero rows persist.
            kabs = []
            for i in range(2):
                kt = constp.tile([P, OC, T], bf16, name=f"kab{i}")
                nc.vector.memset(kt[H:P], 0.0)
                kabs.append(kt)
            # v staging [tk, h] with a persistent ones column at h=64
            vaugs = []
            for i in range(8):
                vt = constp.tile([P, 2, 2, HP1], bf16, name=f"vaug{i}")
                nc.vector.memset(vt[:, :, :, H : H + 1], 1.0)
                vaugs.append(vt)

            for oc in range(nocts):
                b0 = OC * oc
                xt = xpool.tile([P, OC, EC, T], bf16)
                # two half-octet DMAs so compute can start on the first half
                nc.sync.dma_start(xt[:, 0 : OC // 2], xt_d[:, b0 : b0 + OC // 2])
                nc.sync.dma_start(
                    xt[:, OC // 2 : OC], xt_d[:, b0 + OC // 2 : b0 + OC]
                )
                qk_sb = qkpool.tile([P, OC, T], bf16)
                k_sb = kabs[oc % 2]
                o_sb = opool.tile([P, OC, 2, H], bf16)

                # ---- projections (per pair) ----
                for pr in range(OC // 2):
                    s0 = 2 * pr
                    qk_ps = ps_qk.tile([P, 2, T], f32)
                    v_psf = ps_v.tile([P, 2, 2, P], f32)
                    v_ps = v_psf[:, :, :, 0:H]
                    for c in range(EC):
                        nc.tensor.matmul(
                            qk_ps,
                            wqk[:, c, :],
                            xt[:, s0 : s0 + 2, c, :],
                            start=(c == 0),
                            stop=(c == EC - 1),
                        )
                    for s in range(2):
                        for j in range(2):
                            for c in range(EC):
                                nc.tensor.matmul(
                                    v_ps[:, s, j, :],
                                    xt[:, s0 + s, c, j * P : (j + 1) * P],
                                    wv[:, c, :],
                                    start=(c == 0),
                                    stop=(c == EC - 1),
                                )
                    # PSUM -> SBUF bf16; alternate engines to balance load
                    if pr % 2 == 0:
                        nc.scalar.copy(qk_sb[:, s0 : s0 + 2, :], qk_ps)
                    else:
                        nc.vector.tensor_copy(qk_sb[:, s0 : s0 + 2, :], qk_ps)
                    v_aug = vaugs[(oc * (OC // 2) + pr) % 8]
                    nc.vector.tensor_copy(v_aug[:, :, :, 0:H], v_ps)

                # k^T partitions 64:128 -> 0:64, whole octet at once
                nc.sync.dma_start(k_sb[0:H], qk_sb[H:P])
                if _dbg:
                    nc.sync.dma_start(dbg_qk[:, b0 : b0 + OC], qk_sb)

                # ---- attention (per pair) ----
                for pr in range(OC // 2):
                    s0 = 2 * pr
                    v_aug = vaugs[(oc * (OC // 2) + pr) % 8]
                    p_sb = ppool.tile([P, 2, 3 * P], bf16)
                    for s in range(2):
                        s_ps = ps_s.tile([P, 3 * P], f32, name="s_ps")
                        # S^T[tk 0:128, tq 0:256]
                        nc.tensor.matmul(
                            s_ps[:, 0:T],
                            k_sb[:, s0 + s, 0:P],
                            qk_sb[:, s0 + s, :],
                            start=True,
                            stop=True,
                        )
                        # S^T[tk 128:256, tq 128:256]
                        nc.tensor.matmul(
                            s_ps[:, T : 3 * P],
                            k_sb[:, s0 + s, P:T],
                            qk_sb[:, s0 + s, P:T],
                            start=True,
                            stop=True,
                        )
                        nc.scalar.activation(p_sb[:, s, :], s_ps, Exp, scale=0.125)

                    # multiplicative causal mask on the two diagonal blocks
                    _meng = nc.gpsimd if _MASK_ENGINE == "gpsimd" else nc.vector
                    _meng.tensor_tensor(
                        p_sb[:, :, 0:P], p_sb[:, :, 0:P], trilb, Mult
                    )
                    _meng.tensor_tensor(
                        p_sb[:, :, T : 3 * P], p_sb[:, :, T : 3 * P], trilb, Mult
                    )

                    if _dbg:
                        nc.sync.dma_start(
                            dbg_k[:, b0 + s0 : b0 + s0 + 2], k_sb[:, s0 : s0 + 2]
                        )
                        nc.sync.dma_start(
                            dbg_p[:, b0 + s0 : b0 + s0 + 2],
                            p_sb.rearrange("p s c -> p s c"),
                        )
                        nc.sync.dma_start(
                            dbg_v[:, b0 + s0 : b0 + s0 + 2], v_aug
                        )
                    o_ps = ps_o.tile([P, 2, 2, HP1], f32)
                    for s in range(2):
                        nc.tensor.matmul(
                            o_ps[:, s, 0, :],
                            p_sb[:, s, 0:P],
                            v_aug[:, s, 0, :],
                            start=True,
                            stop=True,
                        )
                        nc.tensor.matmul(
                            o_ps[:, s, 1, :],
                            p_sb[:, s, P:T],
                            v_aug[:, s, 0, :],
                            start=True,
                            stop=False,
                        )
                        nc.tensor.matmul(
                            o_ps[:, s, 1, :],
                            p_sb[:, s, T : 3 * P],
                            v_aug[:, s, 1, :],
                            start=False,
                            stop=True,
                        )

                    if _dbg:
                        nc.vector.tensor_copy(
                            _dbgo_sb := rpool.tile([P, 2, 2, HP1], f32, name="dbgo"),
                            o_ps,
                        )
                        nc.sync.dma_start(
                            dbg_o[:, b0 + s0 : b0 + s0 + 2], _dbgo_sb
                        )
                    rec = rpool.tile([P, 2, 2, 1], f32)
                    nc.vector.reciprocal(rec, o_ps[:, :, :, H : H + 1])
                    nc.vector.tensor_tensor(
                        o_sb[:, s0 : s0 + 2, :, :],
                        o_ps[:, :, :, 0:H],
                        rec.to_broadcast([P, 2, 2, H]),
                        Mult,
                    )

                nc.sync.dma_start(out_d[:, b0 : b0 + OC], o_sb)

    nc.compile()
    return nc


def _prep_inputs(x, Wq, Wk, Wv, bpc):
    bf = ml_dtypes.bfloat16
    nb = NCORES * bpc
    x = np.asarray(x, dtype=np.float32)[:nb]
    # [b, t, e] -> [p, b, c, t] with e = c*128 + p  (p-major for the DMA)
    xt = np.ascontiguousarray(
        x.reshape(nb, T, EC, P).transpose(3, 0, 2, 1)
    ).astype(bf)
    wqk = np.concatenate(
        [np.asarray(Wq, np.float32), np.asarray(Wk, np.float32)], axis=1
    )  # [E, 128]
    wqk = np.ascontiguousarray(wqk.reshape(EC, P, P).transpose(1, 0, 2)).astype(bf)
    wv = np.ascontiguousarray(
        np.asarray(Wv, np.float32).reshape(EC, P, H).transpose(1, 0, 2)
    ).astype(bf)
    tril01 = (np.arange(P)[:, None] <= np.arange(P)[None, :]).astype(np.float32)
    tril = tril01.astype(bf)
    per_core = []
    for c in range(NCORES):
        per_core.append(
            {
                "xt": np.ascontiguousarray(xt[:, c * bpc : (c + 1) * bpc]),
                "wqk": wqk,
                "wv": wv,
                "tril": tril,
            }
        )
    return per_core


def kernel(x, Wq, Wk, Wv, _trace=False, _bpc=BPC):
    """Full inputs in, full output out. Shards batch dim over 8 NeuronCores."""
    from concourse import bass_utils

    if _trace:
        _install_ntff_hook()

    key = ("prog", _bpc)
    if key not in _cache:
        _cache[key] = _build_program(_bpc)
    nc = _cache[key]

    in_maps = _prep_inputs(x, Wq, Wk, Wv, _bpc)
    res = bass_utils.run_bass_kernel_spmd(
        nc, in_maps, core_ids=list(range(NCORES)), trace=_trace
    )
    _cache["last_result"] = res
    outs = []
    for r in res.results:
        o = np.asarray(r["out"])  # [P, bpc, 2, H] bf16
        outs.append(
            o.transpose(1, 2, 0, 3).reshape(_bpc, T, H).astype(np.float32)
        )
    return np.concatenate(outs, axis=0)


# revision 11
# speedup vs baseline: 1.1109x; 1.0808x over previous
"""Trainium2 Bass kernel: batched causal single-head self-attention.

Reference computation (per batch b):
    q = x @ Wq; k = x @ Wk; v = x @ Wv          # [T, H] each, contraction over E
    S = (q @ k^T) / sqrt(H)                     # [T, T]
    P = softmax(causal_mask(S), axis=-1)
    out = P @ v                                 # [T, H]

Shapes: x [512, 256, 384] f32, W* [384, 64] f32, out [512, 256, 64] f32.
Sharding: pure data parallel, 64 batches per NeuronCore across 8 cores.

Device algorithm per batch (matmul operands bf16, fp32 PSUM accumulation):
  - host ships xt = x^T per batch ([E, T] layout, E on partitions, p-major
    DRAM so every DMA is one contiguous run per partition).
  - [q^T; k^T] = [Wq|Wk]^T @ xt     (one packed 128-wide stationary, 3
    E-chunks, both batches of a pair as one N=512 moving operand)
  - v        = xt_chunk.T @ Wv      (xt chunks [e,t] as stationary -> v in
    [t, h] layout directly; no transpose anywhere)
  - S^T      = k^T.T @ q^T          ([tk, tq]; lower-left T/4 block skipped)
  - P        = exp(0.125 * S^T)     (ScalarE; no max-subtraction, |s|<~45)
  - P       *= causal 0/1 mask      (GpSimd, only the two diagonal blocks)
  - out_aug[tq, 0:65] = sum_tk P[tk,tq] * [v|1][tk]  (col 64 = softmax
    denominator via the ones column); divide on DVE, store bf16.

The emission is software-pipelined at pair granularity: step g issues
PROJ(g), SCORES(g-4), OUT(g-6) so the in-order per-engine streams never
block on the exp->mask->out dependency chain, and a warm-up burst of
matmuls keeps the PE HAM clock-gate open while the first input DMA lands.
"""

import numpy as np
import ml_dtypes

B, T, E, H = 512, 256, 384, 64
NCORES = 8
BPC = B // NCORES  # 64
P = 128
EC = E // P  # 3
HP1 = H + 1  # 65

_cache: dict = {}


def _install_ntff_hook():
    """Shim antenv.axon_hooks (absent in this image) so run_bass_kernel_spmd
    trace=True can capture NTFF profiles via the axon .so's C ABI."""
    import contextlib
    import ctypes
    import sys
    import types

    if "antenv.axon_hooks" in sys.modules:
        return
    so_path = "/opt/axon/libaxon_pjrt.so"
    lib = ctypes.CDLL(so_path)
    if not hasattr(lib, "axon_start_nrt_profile"):
        return
    lib.axon_start_nrt_profile.argtypes = [
        ctypes.POINTER(ctypes.c_int64),
        ctypes.c_size_t,
    ]
    lib.axon_start_nrt_profile.restype = ctypes.c_int64
    lib.axon_stop_nrt_profile.argtypes = [ctypes.c_char_p]
    lib.axon_stop_nrt_profile.restype = ctypes.c_int64

    @contextlib.contextmanager
    def _hook(output_dir, device_ids):
        import jax

        jax.devices()
        if device_ids:
            ids = (ctypes.c_int64 * len(device_ids))(*device_ids)
            rc = lib.axon_start_nrt_profile(ids, len(device_ids))
        else:
            rc = lib.axon_start_nrt_profile(None, 0)
        if rc != 0:
            raise RuntimeError(f"axon_start_nrt_profile rc={rc}")
        try:
            yield
        finally:
            n = lib.axon_stop_nrt_profile(str(output_dir).encode())
            if n < 0:
                raise RuntimeError(f"axon_stop_nrt_profile rc={n}")
            print(f"profile: {n} file(s) written to {output_dir}", file=sys.stderr)

    mod = types.ModuleType("antenv.axon_hooks")
    _state = {"hook": _hook}
    mod.get_axon_ntff_profile_hook = lambda: _state["hook"]
    mod.set_axon_ntff_profile_hook = lambda h: _state.__setitem__("hook", h)
    sys.modules["antenv.axon_hooks"] = mod


def _build_program(bpc):
    import concourse.bacc as bacc
    import concourse.mybir as mybir
    import concourse.tile as tile

    f32 = mybir.dt.float32
    bf16 = mybir.dt.bfloat16
    Exp = mybir.ActivationFunctionType.Exp
    Mult = mybir.AluOpType.mult

    nc = bacc.Bacc(
        "TRN2",
        target_bir_lowering=False,
        debug=False,
        enable_asserts=False,
        num_devices=NCORES,
    )
    # p-major DRAM layouts: one contiguous run per partition per DMA.
    xt_d = nc.dram_tensor("xt", [P, bpc, EC, T], bf16, kind="ExternalInput").ap()
    wqk_d = nc.dram_tensor("wqk", [P, EC, P], bf16, kind="ExternalInput").ap()
    wv_d = nc.dram_tensor("wv", [P, EC, H], bf16, kind="ExternalInput").ap()
    tril_d = nc.dram_tensor("tril", [P, P], bf16, kind="ExternalInput").ap()
    out_d = nc.dram_tensor("out", [P, bpc, 2, H], bf16, kind="ExternalOutput").ap()

    OC = 8  # batches per octet (DMA granularity)
    PPO = OC // 2  # pairs per octet
    assert bpc % OC == 0
    nocts = bpc // OC
    npairs = bpc // 2
    SC_LAG = 4  # SCORES(g) at step g+SC_LAG (octet projections + shift done)
    OUT_LAG = 6  # OUT(g) at step g+OUT_LAG (exp+mask latency hidden)
    NVAUG = 8

    with tile.TileContext(nc) as tc:
        with (
            tc.tile_pool(name="const", bufs=1) as constp,
            tc.tile_pool(name="xin", bufs=3) as xpool,
            tc.tile_pool(name="qksb", bufs=3) as qkpool,
            tc.tile_pool(name="psb", bufs=4) as ppool,
            tc.tile_pool(name="osb", bufs=3) as opool,
            tc.tile_pool(name="rec", bufs=2) as rpool,
            tc.tile_pool(name="ps_qk", bufs=2, space="PSUM") as ps_qk,
            tc.tile_pool(name="ps_v", bufs=2, space="PSUM") as ps_v,
            tc.tile_pool(name="ps_s", bufs=2, space="PSUM") as ps_s,
            tc.tile_pool(name="ps_o", bufs=2, space="PSUM") as ps_o,
        ):
            wqk = constp.tile([P, EC, P], bf16)
            nc.sync.dma_start(wqk, wqk_d)
            wv = constp.tile([P, EC, H], bf16)
            nc.sync.dma_start(wv, wv_d)
            tril = constp.tile([P, P], bf16)
            nc.sync.dma_start(tril, tril_d)
            trilb = tril[:, None, :].to_broadcast([P, 2, P])

            # k^T staging padded to 128 partitions with zero rows 64:128 so
            # the scores matmuls use full-width stationaries; shift-DMA fills
            # rows 0:64 each octet, the zero rows persist.
            kabs = []
            for i in range(2):
                kt = constp.tile([P, OC, T], bf16, name=f"kab{i}")
                nc.vector.memset(kt[H:P], 0.0)
                kabs.append(kt)
            # v staging [tk, h] with a persistent ones column at h=64
            vaugs = []
            for i in range(NVAUG):
                vt = constp.tile([P, 2, 2, HP1], bf16, name=f"vaug{i}")
                nc.vector.memset(vt[:, :, :, H : H + 1], 1.0)
                vaugs.append(vt)

            # HAM warm-up: keep the PE busy (~4us) while the first x octet
            # streams in, so real matmuls start at the 2.4 GHz clock.
            warm = ps_s.tile([P, 3 * P], f32, name="s_ps")
            for i in range(14):
                nc.tensor.matmul(
                    warm,
                    wqk[:, i % EC, :],
                    wqk.rearrange("p c m -> p (c m)"),
                    start=True,
                    stop=True,
                )

            xts: dict = {}
            qks: dict = {}
            osbs: dict = {}
            psbs: dict = {}

            def ensure_oct(o):
                if o >= nocts or o in xts:
                    return
                b0 = OC * o
                xt = xpool.tile([P, OC, EC, T], bf16)
                nc.sync.dma_start(xt[:, 0 : OC // 2], xt_d[:, b0 : b0 + OC // 2])
                nc.sync.dma_start(
                    xt[:, OC // 2 : OC], xt_d[:, b0 + OC // 2 : b0 + OC]
                )
                xts[o] = xt
                qks[o] = qkpool.tile([P, OC, T], bf16, name="qk_sb")
                osbs[o] = opool.tile([P, OC, 2, H], bf16, name="o_sb")

            def proj(g):
                o, pr = divmod(g, PPO)
                s0 = 2 * pr
                xt, qk_sb = xts[o], qks[o]
                qk_ps = ps_qk.tile([P, 2, T], f32)
                v_psf = ps_v.tile([P, 2, 2, P], f32)
                v_ps = v_psf[:, :, :, 0:H]
                for c in range(EC):
                    nc.tensor.matmul(
                        qk_ps,
                        wqk[:, c, :],
                        xt[:, s0 : s0 + 2, c, :],
                        start=(c == 0),
                        stop=(c == EC - 1),
                    )
                for s in range(2):
                    for j in range(2):
                        for c in range(EC):
                            nc.tensor.matmul(
                                v_ps[:, s, j, :],
                                xt[:, s0 + s, c, j * P : (j + 1) * P],
                                wv[:, c, :],
                                start=(c == 0),
                                stop=(c == EC - 1),
                            )
                if pr % 2 == 0:
                    nc.scalar.copy(qk_sb[:, s0 : s0 + 2, :], qk_ps)
                else:
                    nc.vector.tensor_copy(qk_sb[:, s0 : s0 + 2, :], qk_ps)
                nc.vector.tensor_copy(vaugs[g % NVAUG][:, :, :, 0:H], v_ps)

            def scores(g):
                o, pr = divmod(g, PPO)
                s0 = 2 * pr
                qk_sb, k_sb = qks[o], kabs[o % 2]
                p_sb = ppool.tile([P, 2, 3 * P], bf16, name="p_sb")
                psbs[g] = p_sb
                for s in range(2):
                    s_ps = ps_s.tile([P, 3 * P], f32, name="s_ps")
                    nc.tensor.matmul(
                        s_ps[:, 0:T],
                        k_sb[:, s0 + s, 0:P],
                        qk_sb[:, s0 + s, :],
                        start=True,
                        stop=True,
                    )
                    nc.tensor.matmul(
                        s_ps[:, T : 3 * P],
                        k_sb[:, s0 + s, P:T],
                        qk_sb[:, s0 + s, P:T],
                        start=True,
                        stop=True,
                    )
                    nc.scalar.activation(p_sb[:, s, :], s_ps, Exp, scale=0.125)
                # multiplicative causal mask on the two diagonal blocks
                nc.gpsimd.tensor_tensor(
                    p_sb[:, :, 0:P], p_sb[:, :, 0:P], trilb, Mult
                )
                nc.gpsimd.tensor_tensor(
                    p_sb[:, :, T : 3 * P], p_sb[:, :, T : 3 * P], trilb, Mult
                )

            def out(g):
                o, pr = divmod(g, PPO)
                s0 = 2 * pr
                o_sb = osbs[o]
                v_aug = vaugs[g % NVAUG]
                p_sb = psbs.pop(g)
                o_ps = ps_o.tile([P, 2, 2, HP1], f32)
                for s in range(2):
                    nc.tensor.matmul(
                        o_ps[:, s, 0, :],
                        p_sb[:, s, 0:P],
                        v_aug[:, s, 0, :],
                        start=True,
                        stop=True,
                    )
                    nc.tensor.matmul(
                        o_ps[:, s, 1, :],
                        p_sb[:, s, P:T],
                        v_aug[:, s, 0, :],
                        start=True,
                        stop=False,
                    )
                    nc.tensor.matmul(
                        o_ps[:, s, 1, :],
                        p_sb[:, s, T : 3 * P],
                        v_aug[:, s, 1, :],
                        start=False,
                        stop=True,
                    )
                rec = rpool.tile([P, 2, 2, 1], f32)
                nc.vector.reciprocal(rec, o_ps[:, :, :, H : H + 1])
                nc.vector.tensor_tensor(
                    o_sb[:, s0 : s0 + 2, :, :],
                    o_ps[:, :, :, 0:H],
                    rec.to_broadcast([P, 2, 2, H]),
                    Mult,
                )

            ensure_oct(0)
            for step in range(npairs + OUT_LAG):
                if step < npairs:
                    o, pr = divmod(step, PPO)
                    if pr == 0:
                        ensure_oct(o + 1)
                    proj(step)
                    if pr == PPO - 1:
                        # k^T partitions 64:128 -> 0:64, whole octet at once
                        nc.sync.dma_start(kabs[o % 2][0:H], qks[o][H:P])
                gs = step - SC_LAG
                if 0 <= gs < npairs:
                    scores(gs)
                go = step - OUT_LAG
                if 0 <= go < npairs:
                    out(go)
                    o, pr = divmod(go, PPO)
                    if pr == PPO - 1:
                        nc.sync.dma_start(
                            out_d[:, OC * o : OC * (o + 1)], osbs.pop(o)
                        )

    nc.compile()
    return nc


def _prep_inputs(x, Wq, Wk, Wv, bpc):
    bf = ml_dtypes.bfloat16
    nb = NCORES * bpc
    x = np.asarray(x, dtype=np.float32)[:nb]
    # [b, t, e] -> [p, b, c, t] with e = c*128 + p  (p-major for the DMA)
    xt = np.ascontiguousarray(
        x.reshape(nb, T, EC, P).transpose(3, 0, 2, 1)
    ).astype(bf)
    wqk = np.concatenate(
        [np.asarray(Wq, np.float32), np.asarray(Wk, np.float32)], axis=1
    )  # [E, 128]
    wqk = np.ascontiguousarray(wqk.reshape(EC, P, P).transpose(1, 0, 2)).astype(bf)
    wv = np.ascontiguousarray(
        np.asarray(Wv, np.float32).reshape(EC, P, H).transpose(1, 0, 2)
    ).astype(bf)
    tril01 = (np.arange(P)[:, None] <= np.arange(P)[None, :]).astype(np.float32)
    tril = tril01.astype(bf)
    per_core = []
    for c in range(NCORES):
        per_core.append(
            {
                "xt": np.ascontiguousarray(xt[:, c * bpc : (c + 1) * bpc]),
                "wqk": wqk,
                "wv": wv,
                "tril": tril,
            }
        )
    return per_core


def kernel(x, Wq, Wk, Wv, _trace=False, _bpc=BPC):
    """Full inputs in, full output out. Shards batch dim over 8 NeuronCores."""
    from concourse import bass_utils

    if _trace:
        _install_ntff_hook()

    key = ("prog", _bpc)
    if key not in _cache:
        _cache[key] = _build_program(_bpc)
    nc = _cache[key]

    in_maps = _prep_inputs(x, Wq, Wk, Wv, _bpc)
    res = bass_utils.run_bass_kernel_spmd(
        nc, in_maps, core_ids=list(range(NCORES)), trace=_trace
    )
    _cache["last_result"] = res
    outs = []
    for r in res.results:
        o = np.asarray(r["out"])  # [P, bpc, 2, H] bf16
        outs.append(
            o.transpose(1, 2, 0, 3).reshape(_bpc, T, H).astype(np.float32)
        )
    return np.concatenate(outs, axis=0)


# revision 12
# speedup vs baseline: 1.1250x; 1.0127x over previous
"""Trainium2 Bass kernel: batched causal single-head self-attention.

Reference computation (per batch b):
    q = x @ Wq; k = x @ Wk; v = x @ Wv          # [T, H] each, contraction over E
    S = (q @ k^T) / sqrt(H)                     # [T, T]
    P = softmax(causal_mask(S), axis=-1)
    out = P @ v                                 # [T, H]

Shapes: x [512, 256, 384] f32, W* [384, 64] f32, out [512, 256, 64] f32.
Sharding: pure data parallel, 64 batches per NeuronCore across 8 cores.

Device algorithm per batch (matmul operands bf16, fp32 PSUM accumulation):
  - host ships xt = x^T per batch ([E, T] layout, E on partitions, p-major
    DRAM so every DMA is one contiguous run per partition).
  - [q^T; k^T] = [Wq|Wk]^T @ xt     (one packed 128-wide stationary, 3
    E-chunks, both batches of a pair as one N=512 moving operand)
  - v        = xt_chunk.T @ Wv      (xt chunks [e,t] as stationary -> v in
    [t, h] layout directly; no transpose anywhere)
  - S^T      = k^T.T @ q^T          ([tk, tq]; lower-left T/4 block skipped)
  - P        = exp(0.125 * S^T)     (ScalarE; no max-subtraction, |s|<~45)
  - P       *= causal 0/1 mask      (GpSimd, only the two diagonal blocks)
  - out_aug[tq, 0:65] = sum_tk P[tk,tq] * [v|1][tk]  (col 64 = softmax
    denominator via the ones column); divide on DVE, store bf16.

The emission is software-pipelined at pair granularity: step g issues
PROJ(g), SCORES(g-4), OUT(g-6) so the in-order per-engine streams never
block on the exp->mask->out dependency chain, and a warm-up burst of
matmuls keeps the PE HAM clock-gate open while the first input DMA lands.
"""

import numpy as np
import ml_dtypes

B, T, E, H = 512, 256, 384, 64
NCORES = 8
BPC = B // NCORES  # 64
P = 128
EC = E // P  # 3
HP1 = H + 1  # 65

_cache: dict = {}


def _install_ntff_hook():
    """Shim antenv.axon_hooks (absent in this image) so run_bass_kernel_spmd
    trace=True can capture NTFF profiles via the axon .so's C ABI."""
    import contextlib
    import ctypes
    import sys
    import types

    if "antenv.axon_hooks" in sys.modules:
        return
    so_path = "/opt/axon/libaxon_pjrt.so"
    lib = ctypes.CDLL(so_path)
    if not hasattr(lib, "axon_start_nrt_profile"):
        return
    lib.axon_start_nrt_profile.argtypes = [
        ctypes.POINTER(ctypes.c_int64),
        ctypes.c_size_t,
    ]
    lib.axon_start_nrt_profile.restype = ctypes.c_int64
    lib.axon_stop_nrt_profile.argtypes = [ctypes.c_char_p]
    lib.axon_stop_nrt_profile.restype = ctypes.c_int64

    @contextlib.contextmanager
    def _hook(output_dir, device_ids):
        import jax

        jax.devices()
        if device_ids:
            ids = (ctypes.c_int64 * len(device_ids))(*device_ids)
            rc = lib.axon_start_nrt_profile(ids, len(device_ids))
        else:
            rc = lib.axon_start_nrt_profile(None, 0)
        if rc != 0:
            raise RuntimeError(f"axon_start_nrt_profile rc={rc}")
        try:
            yield
        finally:
            n = lib.axon_stop_nrt_profile(str(output_dir).encode())
            if n < 0:
                raise RuntimeError(f"axon_stop_nrt_profile rc={n}")
            print(f"profile: {n} file(s) written to {output_dir}", file=sys.stderr)

    mod = types.ModuleType("antenv.axon_hooks")
    _state = {"hook": _hook}
    mod.get_axon_ntff_profile_hook = lambda: _state["hook"]
    mod.set_axon_ntff_profile_hook = lambda h: _state.__setitem__("hook", h)
    sys.modules["antenv.axon_hooks"] = mod


def _build_program(bpc):
    import concourse.bacc as bacc
    import concourse.mybir as mybir
    import concourse.tile as tile

    f32 = mybir.dt.float32
    bf16 = mybir.dt.bfloat16
    Exp = mybir.ActivationFunctionType.Exp
    Mult = mybir.AluOpType.mult

    nc = bacc.Bacc(
        "TRN2",
        target_bir_lowering=False,
        debug=False,
        enable_asserts=False,
        num_devices=NCORES,
    )
    # p-major DRAM layouts: one contiguous run per partition per DMA.
    xt_d = nc.dram_tensor("xt", [P, bpc, EC, T], bf16, kind="ExternalInput").ap()
    wqk_d = nc.dram_tensor("wqk", [P, EC, P], bf16, kind="ExternalInput").ap()
    wv_d = nc.dram_tensor("wv", [P, EC, H], bf16, kind="ExternalInput").ap()
    tril_d = nc.dram_tensor("tril", [P, P], bf16, kind="ExternalInput").ap()
    out_d = nc.dram_tensor("out", [P, bpc, 2, H], bf16, kind="ExternalOutput").ap()

    OC = 8  # batches per octet (DMA granularity)
    PPO = OC // 2  # pairs per octet
    assert bpc % OC == 0
    nocts = bpc // OC
    npairs = bpc // 2
    SC_LAG = 5  # SCORES(g) at step g+SC_LAG (octet projections + shift done)
    OUT_LAG = 7  # OUT(g) at step g+OUT_LAG (exp+mask latency hidden)
    NVAUG = 10

    with tile.TileContext(nc) as tc:
        with (
            tc.tile_pool(name="const", bufs=1) as constp,
            tc.tile_pool(name="xin", bufs=3) as xpool,
            tc.tile_pool(name="qksb", bufs=4) as qkpool,
            tc.tile_pool(name="psb", bufs=4) as ppool,
            tc.tile_pool(name="osb", bufs=3) as opool,
            tc.tile_pool(name="rec", bufs=2) as rpool,
            tc.tile_pool(name="ps_qk", bufs=2, space="PSUM") as ps_qk,
            tc.tile_pool(name="ps_v", bufs=2, space="PSUM") as ps_v,
            tc.tile_pool(name="ps_s", bufs=2, space="PSUM") as ps_s,
            tc.tile_pool(name="ps_o", bufs=2, space="PSUM") as ps_o,
        ):
            wqk = constp.tile([P, EC, P], bf16)
            nc.sync.dma_start(wqk, wqk_d)
            wv = constp.tile([P, EC, H], bf16)
            nc.sync.dma_start(wv, wv_d)
            tril = constp.tile([P, P], bf16)
            nc.sync.dma_start(tril, tril_d)
            trilb = tril[:, None, :].to_broadcast([P, 2, P])

            # k^T staging padded to 128 partitions with zero rows 64:128 so
            # the scores matmuls use full-width stationaries; shift-DMA fills
            # rows 0:64 each octet, the zero rows persist.
            kabs = []
            for i in range(2):
                kt = constp.tile([P, OC, T], bf16, name=f"kab{i}")
                nc.vector.memset(kt[H:P], 0.0)
                kabs.append(kt)
            # v staging [tk, h] with a persistent ones column at h=64
            vaugs = []
            for i in range(NVAUG):
                vt = constp.tile([P, 2, 2, HP1], bf16, name=f"vaug{i}")
                nc.vector.memset(vt[:, :, :, H : H + 1], 1.0)
                vaugs.append(vt)

            # HAM warm-up: keep the PE busy (~4us) while the first x octet
            # streams in, so real matmuls start at the 2.4 GHz clock.
            warm = ps_s.tile([P, 3 * P], f32, name="s_ps")
            for i in range(14):
                nc.tensor.matmul(
                    warm,
                    wqk[:, i % EC, :],
                    wqk.rearrange("p c m -> p (c m)"),
                    start=True,
                    stop=True,
                )

            xts: dict = {}
            qks: dict = {}
            osbs: dict = {}
            psbs: dict = {}

            def ensure_oct(o):
                if o >= nocts or o in xts:
                    return
                b0 = OC * o
                xt = xpool.tile([P, OC, EC, T], bf16)
                nc.sync.dma_start(xt[:, 0 : OC // 2], xt_d[:, b0 : b0 + OC // 2])
                nc.sync.dma_start(
                    xt[:, OC // 2 : OC], xt_d[:, b0 + OC // 2 : b0 + OC]
                )
                xts[o] = xt
                qks[o] = qkpool.tile([P, OC, T], bf16, name="qk_sb")

            def proj_mms(g):
                o, pr = divmod(g, PPO)
                s0 = 2 * pr
                xt = xts[o]
                qk_ps = ps_qk.tile([P, 2, T], f32)
                v_psf = ps_v.tile([P, 2, 2, P], f32)
                v_ps = v_psf[:, :, :, 0:H]
                qk_ops = [
                    (lambda c=c: nc.tensor.matmul(
                        qk_ps,
                        wqk[:, c, :],
                        xt[:, s0 : s0 + 2, c, :],
                        start=(c == 0),
                        stop=(c == EC - 1),
                    ))
                    for c in range(EC)
                ]
                v_ops = [
                    (lambda s=s, j=j, c=c: nc.tensor.matmul(
                        v_ps[:, s, j, :],
                        xt[:, s0 + s, c, j * P : (j + 1) * P],
                        wv[:, c, :],
                        start=(c == 0),
                        stop=(c == EC - 1),
                    ))
                    for s in range(2)
                    for j in range(2)
                    for c in range(EC)
                ]
                return qk_ps, v_ps, qk_ops, v_ops

            def proj_copies(g, qk_ps, v_ps):
                o, pr = divmod(g, PPO)
                s0 = 2 * pr
                qk_sb = qks[o]
                if pr % 2 == 0:
                    nc.scalar.copy(qk_sb[:, s0 : s0 + 2, :], qk_ps)
                else:
                    nc.vector.tensor_copy(qk_sb[:, s0 : s0 + 2, :], qk_ps)
                nc.vector.tensor_copy(vaugs[g % NVAUG][:, :, :, 0:H], v_ps)

            def scores_mms(g):
                o, pr = divmod(g, PPO)
                s0 = 2 * pr
                qk_sb, k_sb = qks[o], kabs[o % 2]
                p_sb = ppool.tile([P, 2, 3 * P], bf16, name="p_sb")
                psbs[g] = p_sb
                s_pss = []
                ops = []
                for s in range(2):
                    s_ps = ps_s.tile([P, 3 * P], f32, name="s_ps")
                    s_pss.append(s_ps)
                    ops.append(lambda s=s, s_ps=s_ps: nc.tensor.matmul(
                        s_ps[:, 0:T],
                        k_sb[:, s0 + s, 0:P],
                        qk_sb[:, s0 + s, :],
                        start=True,
                        stop=True,
                    ))
                    ops.append(lambda s=s, s_ps=s_ps: nc.tensor.matmul(
                        s_ps[:, T : 3 * P],
                        k_sb[:, s0 + s, P:T],
                        qk_sb[:, s0 + s, P:T],
                        start=True,
                        stop=True,
                    ))
                return p_sb, s_pss, ops

            def scores_post(g, p_sb, s_pss, s):
                nc.scalar.activation(p_sb[:, s, :], s_pss[s], Exp, scale=0.125)

            def scores_mask(g, p_sb):
                # multiplicative causal mask on the two diagonal blocks
                nc.gpsimd.tensor_tensor(
                    p_sb[:, :, 0:P], p_sb[:, :, 0:P], trilb, Mult
                )
                nc.gpsimd.tensor_tensor(
                    p_sb[:, :, T : 3 * P], p_sb[:, :, T : 3 * P], trilb, Mult
                )

            def out_mms(g):
                o, pr = divmod(g, PPO)
                if pr == 0 and o not in osbs:
                    osbs[o] = opool.tile([P, OC, 2, H], bf16, name="o_sb")
                v_aug = vaugs[g % NVAUG]
                p_sb = psbs.pop(g)
                o_ps = ps_o.tile([P, 2, 2, HP1], f32)
                ops = []
                for s in range(2):
                    ops.append(lambda s=s: nc.tensor.matmul(
                        o_ps[:, s, 0, :],
                        p_sb[:, s, 0:P],
                        v_aug[:, s, 0, :],
                        start=True,
                        stop=True,
                    ))
                    ops.append(lambda s=s: nc.tensor.matmul(
                        o_ps[:, s, 1, :],
                        p_sb[:, s, P:T],
                        v_aug[:, s, 0, :],
                        start=True,
                        stop=False,
                    ))
                    ops.append(lambda s=s: nc.tensor.matmul(
                        o_ps[:, s, 1, :],
                        p_sb[:, s, T : 3 * P],
                        v_aug[:, s, 1, :],
                        start=False,
                        stop=True,
                    ))
                return o_ps, ops

            def out_post(g, o_ps):
                o, pr = divmod(g, PPO)
                s0 = 2 * pr
                rec = rpool.tile([P, 2, 2, 1], f32)
                nc.vector.reciprocal(rec, o_ps[:, :, :, H : H + 1])
                nc.vector.tensor_tensor(
                    osbs[o][:, s0 : s0 + 2, :, :],
                    o_ps[:, :, :, 0:H],
                    rec.to_broadcast([P, 2, 2, H]),
                    Mult,
                )

            def interleave(long_ops, short_ops):
                # alternate long (N>=256) and short matmuls so short-op
                # LDWEIGHTS hide under the long moving streams
                out = []
                li, si = 0, 0
                while li < len(long_ops) or si < len(short_ops):
                    if li < len(long_ops):
                        out.append(long_ops[li]); li += 1
                    if si < len(short_ops):
                        out.append(short_ops[si]); si += 1
                return out

            ensure_oct(0)
            for step in range(npairs + OUT_LAG):
                g = step if step < npairs else None
                gs = step - SC_LAG if 0 <= step - SC_LAG < npairs else None
                go = step - OUT_LAG if 0 <= step - OUT_LAG < npairs else None

                if g is not None and g % PPO == 0:
                    ensure_oct(g // PPO + 1)

                qk_ops, v_ops, sc_ops, o_ops = [], [], [], []
                if g is not None:
                    qk_ps, v_ps, qk_ops, v_ops = proj_mms(g)
                if gs is not None:
                    p_sb, s_pss, sc_ops = scores_mms(gs)
                if go is not None:
                    o_ps, o_ops = out_mms(go)

                # PE stream: long qk/scores MMs alternated with short out MMs,
                # then the LDW-bound v section
                for op in interleave(qk_ops + sc_ops, o_ops):
                    op()
                if gs is not None:
                    scores_post(gs, p_sb, s_pss, 0)
                    scores_post(gs, p_sb, s_pss, 1)
                for op in v_ops:
                    op()

                if go is not None:
                    out_post(go, o_ps)
                if g is not None:
                    proj_copies(g, qk_ps, v_ps)
                if gs is not None:
                    scores_mask(gs, p_sb)

                # half-octet k-shift as soon as 2 pairs of qk are staged
                if g is not None and g % 2 == 1:
                    o, pr = divmod(g, PPO)
                    h0 = (pr - 1) * 2
                    nc.sync.dma_start(
                        kabs[o % 2][0:H, h0 : h0 + 4], qks[o][H:P, h0 : h0 + 4]
                    )
                # half-octet out store
                if go is not None and go % 2 == 1:
                    o, pr = divmod(go, PPO)
                    h0 = (pr - 1) * 2
                    nc.sync.dma_start(
                        out_d[:, OC * o + h0 : OC * o + h0 + 4],
                        osbs[o][:, h0 : h0 + 4],
                    )
                    if pr == PPO - 1:
                        osbs.pop(o)

    nc.compile()
    return nc


def _prep_inputs(x, Wq, Wk, Wv, bpc):
    bf = ml_dtypes.bfloat16
    nb = NCORES * bpc
    x = np.asarray(x, dtype=np.float32)[:nb]
    # [b, t, e] -> [p, b, c, t] with e = c*128 + p  (p-major for the DMA)
    xt = np.ascontiguousarray(
        x.reshape(nb, T, EC, P).transpose(3, 0, 2, 1)
    ).astype(bf)
    wqk = np.concatenate(
        [np.asarray(Wq, np.float32), np.asarray(Wk, np.float32)], axis=1
    )  # [E, 128]
    wqk = np.ascontiguousarray(wqk.reshape(EC, P, P).transpose(1, 0, 2)).astype(bf)
    wv = np.ascontiguousarray(
        np.asarray(Wv, np.float32).reshape(EC, P, H).transpose(1, 0, 2)
    ).astype(bf)
    tril01 = (np.arange(P)[:, None] <= np.arange(P)[None, :]).astype(np.float32)
    tril = tril01.astype(bf)
    per_core = []
    for c in range(NCORES):
        per_core.append(
            {
                "xt": np.ascontiguousarray(xt[:, c * bpc : (c + 1) * bpc]),
                "wqk": wqk,
                "wv": wv,
                "tril": tril,
            }
        )
    return per_core


def kernel(x, Wq, Wk, Wv, _trace=False, _bpc=BPC):
    """Full inputs in, full output out. Shards batch dim over 8 NeuronCores."""
    from concourse import bass_utils

    if _trace:
        _install_ntff_hook()

    key = ("prog", _bpc)
    if key not in _cache:
        _cache[key] = _build_program(_bpc)
    nc = _cache[key]

    in_maps = _prep_inputs(x, Wq, Wk, Wv, _bpc)
    res = bass_utils.run_bass_kernel_spmd(
        nc, in_maps, core_ids=list(range(NCORES)), trace=_trace
    )
    _cache["last_result"] = res
    outs = []
    for r in res.results:
        o = np.asarray(r["out"])  # [P, bpc, 2, H] bf16
        outs.append(
            o.transpose(1, 2, 0, 3).reshape(_bpc, T, H).astype(np.float32)
        )
    return np.concatenate(outs, axis=0)
